# revision 17
# baseline (speedup 1.0000x reference)
"""DensityGuidedCompressor Trainium2 kernel.

Problem: B=8, N=4096, D=1024, H=1024, NQ=64, TOPK=1024.
  K = X @ key_w + key_b                       [B,N,H]
  s = (query_embed @ K^T)/sqrt(H) + db[n]     [B,NQ,N]
  w = softmax(s, axis=-1); imp = max_q w      [B,N]
  idx = sort(top_k(imp, 1024))                [B,1024]
  out = X[idx]                                [B,1024,D]

Strategy (data-parallel, one batch element per NeuronCore):

Math reductions (exact in real arithmetic, fp32 on device):
  * key_b shifts every score of a q-row uniformly -> cancels in softmax and
    in importance; dropped.
  * q @ K^T = (q @ key_w^T) @ X^T: precompute QW = query_embed @ key_w^T / 32
    on host (param-only transform, 64x1024) -> the big matmul contracts over
    D once instead of computing K at all (16x fewer FLOPs).
  * ranking by imp[n] = max_q softmax(s)[q,n] == ranking by
    g[n] = max_q (s[q,n] - C_q), C_q = logsumexp_n s[q,n]  (exp is monotone).
  * density MLP with b1==0 and d>0 collapses to db = alpha*d + b2 with
    alpha = sum relu(w1)*w2 (host-checked; exact fallback: host computes db).

Device pipeline per core (all selection + gather on device):
  1. S[64,4096] = QWT^T @ XTp in PSUM (fp32 matmuls, 8x8 tiles of K=128,N=512)
  2. SF = S + db_rep; z_q = sum_n exp(SF) (ACT accum); C = ln(z)
  3. SC = SF - C; g = partition_all_reduce_max over the 64 q rows -> [1,4096]
  4. exact 1024th-largest threshold: 4 rounds of 128-way counting grid
     (PE broadcast matmul for the per-partition threshold vector, DVE
     is_ge+accum counts on a partition-broadcast copy of g, PE ones-matmul
     partition reduction, scalar update of [lo,w] on partition 0)
  5. mask = g >= t; masked iota of ORIGINAL token ids; gpsimd sparse_gather
     stream-compacts the 1024 selected ids in ascending order
  6. gpsimd dma_gather pulls the 1024 selected X rows (4KB each) from HBM,
     one DMA writes them back to OUT in row order.

Token permutation: device free-dim position k holds original token
n = (k%256)*16 + k//256 so that the [1,4096] g row splits contiguously into
the [16,256] (partition-minor) layout sparse_gather scans, with the scan
order equal to ascending original token id. XT columns and densities are
permuted on host; the masked-iota constant carries original ids.
"""

import numpy as np

B, N, D, H, NQ = 8, 4096, 1024, 1024, 64
TOPK = 1024
NCHUNK = 512          # matmul moving-operand free size (fp32 max)
NC_COUNT = 8          # cores
ROUNDS = 4            # threshold grid refinement rounds (64-ary)
LO0, HI0 = -16.0, 2.0  # conservative initial g bracket (g in ~[-12,-4] for
                       # unit-scale inputs; 18/64^4 = 1.1e-6 bracket width vs
                       # observed min rank-1024/1025 gap 5.8e-5)


def _build_bass():
    import concourse.bacc as bacc
    import concourse.mybir as mybir
    import concourse.tile as tile
    import concourse.bass as bass
    from concourse import bass_isa

    dt = mybir.dt
    ALU = mybir.AluOpType
    AF = mybir.ActivationFunctionType

    nc = bacc.Bacc("TRN2", target_bir_lowering=False, debug=False)

    XTP = nc.dram_tensor("XTP", [D, N], dt.float32, kind="ExternalInput")
    X = nc.dram_tensor("X", [N, D], dt.float32, kind="ExternalInput")
    DENS = nc.dram_tensor("DENS", [1, N], dt.float32, kind="ExternalInput")
    QWT = nc.dram_tensor("QWT", [D, NQ], dt.float32, kind="ExternalInput")
    DCOEF = nc.dram_tensor("DCOEF", [1, 2], dt.float32, kind="ExternalInput")
    IOTA16 = nc.dram_tensor("IOTA16", [16, 256], dt.float32, kind="ExternalInput")
    IOTAC = nc.dram_tensor("IOTAC", [64, 1], dt.float32, kind="ExternalInput")
    STATE0 = nc.dram_tensor("STATE0", [1, 2], dt.float32, kind="ExternalInput")
    J64 = nc.dram_tensor("J64", [64, 64], dt.float32, kind="ExternalInput")
    ID16 = nc.dram_tensor("ID16", [16, 16], dt.float32, kind="ExternalInput")
    OUT = nc.dram_tensor("OUT", [TOPK, D], dt.float32, kind="ExternalOutput")

    NCH = N // NCHUNK   # 8 n-chunks
    DCH = D // 128      # 8 d-chunks

    with tile.TileContext(nc) as tc:
        with tc.tile_pool(name="consts", bufs=1) as cpool, \
             tc.tile_pool(name="xtp", bufs=3) as xpool, \
             tc.tile_pool(name="work", bufs=1) as wpool, \
             tc.tile_pool(name="small", bufs=2) as spool, \
             tc.tile_pool(name="gath", bufs=2) as gpool:

            # ---- constants / params ----
            qwt = cpool.tile([128, DCH, NQ], dt.float32)
            nc.sync.dma_start(qwt[:], QWT.ap().rearrange("(c p) q -> p c q", c=DCH, p=128))
            dens = cpool.tile([1, N], dt.float32)
            nc.sync.dma_start(dens[:], DENS.ap())
            dcoef = cpool.tile([1, 2], dt.float32)
            nc.sync.dma_start(dcoef[:], DCOEF.ap())
            iota16 = cpool.tile([16, 256], dt.float32)
            nc.sync.dma_start(iota16[:], IOTA16.ap())
            iotac = cpool.tile([64, 1], dt.float32)
            nc.sync.dma_start(iotac[:], IOTAC.ap())
            j64 = cpool.tile([64, 64], dt.float32)
            nc.sync.dma_start(j64[:], J64.ap())
            id16 = cpool.tile([16, 16], dt.float32)
            nc.sync.dma_start(id16[:], ID16.ap())

            # ---- density bias, replicated across the 64 q partitions ----
            dens_rep = wpool.tile([NQ, N], dt.float32)
            nc.gpsimd.partition_broadcast(dens_rep[:], dens[:])
            db_rep = wpool.tile([NQ, N], dt.float32)
            # db = alpha*dens + beta ; alpha/beta as per-partition scalars
            alpha_rep = cpool.tile([NQ, 1], dt.float32)
            beta_rep = cpool.tile([NQ, 1], dt.float32)
            nc.gpsimd.partition_broadcast(alpha_rep[:], dcoef[0:1, 0:1])
            nc.gpsimd.partition_broadcast(beta_rep[:], dcoef[0:1, 1:2])
            nc.vector.tensor_scalar(out=db_rep[:], in0=dens_rep[:],
                                    scalar1=alpha_rep[:], scalar2=beta_rep[:],
                                    op0=ALU.mult, op1=ALU.add)

            # ---- scores + logsumexp ----
            sf = wpool.tile([NQ, N], dt.float32)
            z8 = spool.tile([NQ, NCH], dt.float32, tag="z8")
            with tc.tile_pool(name="psS", bufs=1, space="PSUM") as psS, \
                 tc.tile_pool(name="exps", bufs=2) as epool:
                s_tiles = []
                for c in range(NCH):
                    s_tiles.append(psS.tile([NQ, NCHUNK], dt.float32, tag=f"S{c}", name=f"S{c}"))
                xt_tiles = []
                for d in range(DCH):
                    xt = xpool.tile([128, N], dt.float32, tag="xt")
                    nc.sync.dma_start(xt[:], XTP.ap()[d * 128:(d + 1) * 128, :])
                    xt_tiles.append(xt)
                    for c in range(NCH):
                        nc.tensor.matmul(s_tiles[c][:],
                                         qwt[:, d, :],
                                         xt[:, c * NCHUNK:(c + 1) * NCHUNK],
                                         start=(d == 0), stop=(d == DCH - 1))
                for c in range(NCH):
                    cs = slice(c * NCHUNK, (c + 1) * NCHUNK)
                    nc.vector.tensor_tensor(sf[:, cs], s_tiles[c][:],
                                            db_rep[:, cs], op=ALU.add)
                    e = epool.tile([NQ, NCHUNK], dt.float32, tag="e")
                    nc.scalar.activation(e[:], sf[:, cs], AF.Exp,
                                         accum_out=z8[:, c:c + 1])

            zs = spool.tile([NQ, 1], dt.float32, tag="zs")
            nc.vector.tensor_reduce(zs[:], z8[:], axis=mybir.AxisListType.X,
                                    op=ALU.add)
            cq = spool.tile([NQ, 1], dt.float32, tag="cq")
            nc.scalar.activation(cq[:], zs[:], AF.Ln)

            # ---- g = max_q (sf - C) ----
            nc.vector.tensor_scalar(out=sf[:], in0=sf[:], scalar1=cq[:],
                                    scalar2=None, op0=ALU.subtract)
            # fold 64 -> 32 -> 16 q-rows: partition-shifted copies go through
            # PSUM (DVE cannot mix two SBUF operands at different bases)
            with tc.tile_pool(name="psF", bufs=1, space="PSUM") as psF:
                fold = psF.tile([32, N], dt.float32, tag="fold")
                nc.scalar.copy(fold[:], sf[32:64, :])
                nc.vector.tensor_tensor(sf[0:32, :], sf[0:32, :],
                                        fold[:], op=ALU.max)
            ar = wpool.tile([32, N], dt.float32)
            nc.gpsimd.partition_all_reduce(ar[:], sf[0:32, :], channels=32,
                                           reduce_op=bass_isa.ReduceOp.max)

            # ---- exact top-1024 threshold search (64-ary grid) ----
            # every partition of `ar` already holds the full g row; round
            # state [lo, w] is kept replicated across 64 partitions so the
            # only cross-partition step is an all-reduce-add of the 64
            # bracket indicators.
            strep = spool.tile([64, 2], dt.float32, tag="strep")
            st0 = spool.tile([1, 2], dt.float32, tag="st0")
            nc.sync.dma_start(st0[:], STATE0.ap())
            nc.gpsimd.partition_broadcast(strep[:], st0[:])
            lo_rep = strep[:, 0:1]
            w_rep = strep[:, 1:2]
            gx = wpool.tile([64, N], dt.float32)
            nc.gpsimd.partition_broadcast(gx[:], ar[0:1, :])
            scratch = wpool.tile([64, N], dt.float32)
            thr = spool.tile([64, 1], dt.float32, tag="thr")
            cnt = spool.tile([64, 1], dt.float32, tag="cnt")
            cge = spool.tile([64, 1], dt.float32, tag="cge")
            # thr_p = lo + (p+1) * w
            nc.vector.scalar_tensor_tensor(out=thr[:], in0=iotac[:],
                                           scalar=w_rep, in1=lo_rep,
                                           op0=ALU.mult, op1=ALU.add)
            with tc.tile_pool(name="psT", bufs=2, space="PSUM") as psT:
                for r in range(ROUNDS):
                    nc.vector.tensor_scalar(out=scratch[:], in0=gx[:],
                                            scalar1=thr[:], scalar2=0.0,
                                            op0=ALU.is_ge, op1=ALU.add,
                                            accum_out=cnt[:])
                    nc.vector.tensor_scalar(out=cge[:], in0=cnt[:],
                                            scalar1=float(TOPK), scalar2=None,
                                            op0=ALU.is_ge)
                    # replicated partition-sum of the 64 indicators via all-ones
                    # matmul (keeps gpsimd free for the library preload)
                    psr = psT.tile([64, 1], dt.float32, tag="psr", name=f"psr{r}")
                    nc.tensor.matmul(psr[:], j64[:], cge[:], start=True, stop=True)
                    # lo += pstar * w ; w /= 64 ; thr = lo + (p+1) * w
                    nc.vector.scalar_tensor_tensor(out=lo_rep, in0=psr[:],
                                                   scalar=w_rep, in1=lo_rep,
                                                   op0=ALU.mult, op1=ALU.add)
                    nc.vector.tensor_scalar(out=w_rep, in0=w_rep,
                                            scalar1=1.0 / 64.0, scalar2=None,
                                            op0=ALU.mult)
                    if r < ROUNDS - 1:
                        nc.vector.scalar_tensor_tensor(out=thr[:], in0=iotac[:],
                                                       scalar=w_rep, in1=lo_rep,
                                                       op0=ALU.mult, op1=ALU.add)
                # final threshold t = lo (replicated in lo_rep)

                # ---- compaction: masked iota of ids -> sparse_gather ----
                g16 = spool.tile([16, 256], dt.float32, tag="g16")
                nc.sync.dma_start(g16[:], ar[0:1, :].rearrange("o (r m) -> o r m",
                                                               r=16, m=256))
                mge = spool.tile([16, 256], dt.float32, tag="mge")
                nc.vector.tensor_scalar(out=mge[:], in0=g16[:],
                                        scalar1=lo_rep[0:16, :],
                                        scalar2=None, op0=ALU.is_ge)
                m16 = spool.tile([16, 256], dt.float32, tag="m16")
                nc.vector.tensor_tensor(m16[:], mge[:], iota16[:], op=ALU.mult)
                nc.vector.tensor_scalar(out=m16[:], in0=m16[:], scalar1=-1.0,
                                        scalar2=None, op0=ALU.add)
                comp = spool.tile([16, TOPK // 16], dt.float32, tag="comp")
                nfound = spool.tile([1, 1], dt.uint32, tag="nf")
                nc.gpsimd.sparse_gather(comp[:], m16[:], num_found=nfound[:])

                # ---- selected ids to [128, 8] int32 (k = 8p + c order) ----
                ct = psT.tile([64, 16], dt.float32, tag="ct")
                nc.tensor.transpose(ct[:], comp[:], id16[:])
                cti = spool.tile([64, 16], dt.int32, tag="cti")
                nc.vector.tensor_copy(cti[:], ct[:])
                ctib = spool.tile([128, 8], dt.int32, tag="ctib")
                nc.sync.dma_start(
                    ctib[:],
                    cti[:].rearrange("p (b c) -> p b c", b=2, c=8))
            for h in range(4):
                gt = gpool.tile([128, 2, D], dt.float32, tag="gt", name=f"gt{h}")
                for j in range(2):
                    f = 2 * h + j
                    nc.gpsimd.indirect_dma_start(
                        out=gt[:, j, :], out_offset=None, in_=X.ap(),
                        in_offset=bass.IndirectOffsetOnAxis(ap=ctib[:, f:f + 1],
                                                            axis=0))
                dst = OUT.ap().rearrange("(p f) d -> p f d", p=128,
                                         f=8)[:, 2 * h:2 * (h + 1), :]
                nc.sync.dma_start(dst, gt[:])
    nc.compile()
    return nc


_NC_CACHE = None


def _get_nc():
    global _NC_CACHE
    if _NC_CACHE is None:
        _NC_CACHE = _build_bass()
    return _NC_CACHE


def kernel(token_features, token_densities, query_embed,
           key_w, key_b, de_w1, de_b1, de_w2, de_b2):
    from concourse import bass_utils

    X = np.ascontiguousarray(np.asarray(token_features, dtype=np.float32))
    dens = np.asarray(token_densities, dtype=np.float32)
    Q64 = np.asarray(query_embed, dtype=np.float64)
    kw64 = np.asarray(key_w, dtype=np.float64)
    w1 = np.asarray(de_w1, dtype=np.float64)
    b1 = np.asarray(de_b1, dtype=np.float64)
    w2 = np.asarray(de_w2, dtype=np.float64)
    b2 = np.asarray(de_b2, dtype=np.float64)

    # QWT[d, q] = (query_embed @ key_w^T / sqrt(H))^T  (key_b cancels in softmax)
    QWT = ((Q64 @ kw64.T) / np.sqrt(np.float64(H))).T.astype(np.float32)
    QWT = np.ascontiguousarray(QWT)

    # density bias: exact linear collapse when b1 == 0 and d > 0, else host MLP
    linear_ok = np.all(b1 == 0.0) and np.all(dens > 0.0)
    if linear_ok:
        alpha = float(np.maximum(w1[0], 0.0) @ w2[:, 0])
        beta = float(b2[0])
        dens_dev = dens                       # device computes alpha*d + beta
    else:
        hm = np.maximum(dens[..., None].astype(np.float64) @ w1 + b1, 0.0)
        dens_dev = ((hm @ w2 + b2)[..., 0]).astype(np.float32)  # = db itself
        alpha, beta = 1.0, 0.0

    # device token permutation: position k holds token n = (k%256)*16 + k//256
    # XTp = X^T with columns permuted; built as reshape/transpose
    perm_cols = lambda a: np.ascontiguousarray(
        a.reshape(a.shape[0], 256, 16).transpose(0, 2, 1).reshape(a.shape[0], N))

    iota16 = (np.arange(256, dtype=np.float32)[None, :] * 16.0
              + np.arange(16, dtype=np.float32)[:, None] + 1.0)  # original id + 1
    iotac = (1.0 + np.arange(64, dtype=np.float32)).reshape(64, 1)
    w0 = (HI0 - LO0) / 64.0
    state0 = np.array([[LO0, w0]], np.float32)
    j64 = np.ones((64, 64), np.float32)
    ident16 = np.eye(16, dtype=np.float32)
    dcoefs = np.array([[alpha, beta]], np.float32)

    nc = _get_nc()
    in_maps = []
    for b in range(B):
        XT = np.ascontiguousarray(X[b].T)                     # [D, N]
        in_maps.append({
            "XTP": perm_cols(XT),
            "X": X[b],
            "DENS": perm_cols(dens_dev[b][None, :]),
            "QWT": QWT,
            "DCOEF": dcoefs,
            "IOTA16": iota16,
            "IOTAC": iotac,
            "STATE0": state0,
            "J64": j64,
            "ID16": ident16,
        })

    global _LAST_IN_MAPS
    _LAST_IN_MAPS = in_maps
    res = bass_utils.run_bass_kernel_spmd(nc, in_maps, core_ids=list(range(NC_COUNT)))
    out = np.stack([res.results[b]["OUT"] for b in range(B)])
    return out.astype(np.float32)


_LAST_IN_MAPS = None


# revision 18
# speedup vs baseline: 1.1283x; 1.1283x over previous
"""DensityGuidedCompressor Trainium2 kernel.

Problem: B=8, N=4096, D=1024, H=1024, NQ=64, TOPK=1024.
  K = X @ key_w + key_b                       [B,N,H]
  s = (query_embed @ K^T)/sqrt(H) + db[n]     [B,NQ,N]
  w = softmax(s, axis=-1); imp = max_q w      [B,N]
  idx = sort(top_k(imp, 1024))                [B,1024]
  out = X[idx]                                [B,1024,D]

Strategy (data-parallel, one batch element per NeuronCore):

Math reductions (exact in real arithmetic, fp32 on device):
  * key_b shifts every score of a q-row uniformly -> cancels in softmax and
    in importance; dropped.
  * q @ K^T = (q @ key_w^T) @ X^T: precompute QW = query_embed @ key_w^T / 32
    on host (param-only transform, 64x1024) -> the big matmul contracts over
    D once instead of computing K at all (16x fewer FLOPs).
  * ranking by imp[n] = max_q softmax(s)[q,n] == ranking by
    g[n] = max_q (s[q,n] - C_q), C_q = logsumexp_n s[q,n]  (exp is monotone).
  * density MLP with b1==0 and d>0 collapses to db = alpha*d + b2 with
    alpha = sum relu(w1)*w2 (host-checked; exact fallback: host computes db).

Device pipeline per core (all selection + gather on device):
  1. S[64,4096] = QWT^T @ XTp in PSUM (fp32 matmuls, 8x8 tiles of K=128,N=512)
  2. SF = S + db_rep; z_q = sum_n exp(SF) (ACT accum); C = ln(z)
  3. SC = SF - C; g = partition_all_reduce_max over the 64 q rows -> [1,4096]
  4. exact 1024th-largest threshold: 4 rounds of 128-way counting grid
     (PE broadcast matmul for the per-partition threshold vector, DVE
     is_ge+accum counts on a partition-broadcast copy of g, PE ones-matmul
     partition reduction, scalar update of [lo,w] on partition 0)
  5. mask = g >= t; masked iota of ORIGINAL token ids; gpsimd sparse_gather
     stream-compacts the 1024 selected ids in ascending order
  6. gpsimd dma_gather pulls the 1024 selected X rows (4KB each) from HBM,
     one DMA writes them back to OUT in row order.

Token permutation: device free-dim position k holds original token
n = (k%256)*16 + k//256 so that the [1,4096] g row splits contiguously into
the [16,256] (partition-minor) layout sparse_gather scans, with the scan
order equal to ascending original token id. XT columns and densities are
permuted on host; the masked-iota constant carries original ids.
"""

import numpy as np

B, N, D, H, NQ = 8, 4096, 1024, 1024, 64
TOPK = 1024
NCHUNK = 512          # matmul moving-operand free size (fp32 max)
NC_COUNT = 8          # cores
ROUNDS = 4            # threshold grid refinement rounds (64-ary)
LO0, HI0 = -16.0, 2.0  # conservative initial g bracket (g in ~[-12,-4] for
                       # unit-scale inputs; 18/64^4 = 1.1e-6 bracket width vs
                       # observed min rank-1024/1025 gap 5.8e-5)


def _build_bass():
    import concourse.bacc as bacc
    import concourse.mybir as mybir
    import concourse.tile as tile
    import concourse.bass as bass
    from concourse import bass_isa

    dt = mybir.dt
    ALU = mybir.AluOpType
    AF = mybir.ActivationFunctionType

    nc = bacc.Bacc("TRN2", target_bir_lowering=False, debug=False)

    XTP = nc.dram_tensor("XTP", [D, N], dt.float32, kind="ExternalInput")
    X = nc.dram_tensor("X", [N, D], dt.float32, kind="ExternalInput")
    DENS = nc.dram_tensor("DENS", [1, N], dt.float32, kind="ExternalInput")
    QWT = nc.dram_tensor("QWT", [D, NQ], dt.float32, kind="ExternalInput")
    DCOEF = nc.dram_tensor("DCOEF", [1, 2], dt.float32, kind="ExternalInput")
    IOTA16 = nc.dram_tensor("IOTA16", [16, 256], dt.float32, kind="ExternalInput")
    IOTAC = nc.dram_tensor("IOTAC", [64, 1], dt.float32, kind="ExternalInput")
    STATE0 = nc.dram_tensor("STATE0", [1, 2], dt.float32, kind="ExternalInput")
    J64 = nc.dram_tensor("J64", [64, 64], dt.float32, kind="ExternalInput")
    ID16 = nc.dram_tensor("ID16", [16, 16], dt.float32, kind="ExternalInput")
    OUT = nc.dram_tensor("OUT", [TOPK, D], dt.float32, kind="ExternalOutput")

    NCH = N // NCHUNK   # 8 n-chunks
    DCH = D // 128      # 8 d-chunks

    with tile.TileContext(nc) as tc:
        with tc.tile_pool(name="consts", bufs=1) as cpool, \
             tc.tile_pool(name="xtp", bufs=3) as xpool, \
             tc.tile_pool(name="work", bufs=1) as wpool, \
             tc.tile_pool(name="small", bufs=2) as spool, \
             tc.tile_pool(name="gath", bufs=4) as gpool:

            # ---- constants / params ----
            qwt = cpool.tile([128, DCH, NQ], dt.float32)
            nc.sync.dma_start(qwt[:], QWT.ap().rearrange("(c p) q -> p c q", c=DCH, p=128))
            dens = cpool.tile([1, N], dt.float32)
            nc.sync.dma_start(dens[:], DENS.ap())
            dcoef = cpool.tile([1, 2], dt.float32)
            nc.sync.dma_start(dcoef[:], DCOEF.ap())
            iota16 = cpool.tile([16, 256], dt.float32)
            nc.sync.dma_start(iota16[:], IOTA16.ap())
            iotac = cpool.tile([64, 1], dt.float32)
            nc.sync.dma_start(iotac[:], IOTAC.ap())
            j64 = cpool.tile([64, 64], dt.float32)
            nc.sync.dma_start(j64[:], J64.ap())
            id16 = cpool.tile([16, 16], dt.float32)
            nc.sync.dma_start(id16[:], ID16.ap())

            # ---- density bias, replicated across the 64 q partitions ----
            dens_rep = wpool.tile([NQ, N], dt.float32)
            nc.gpsimd.partition_broadcast(dens_rep[:], dens[:])
            db_rep = wpool.tile([NQ, N], dt.float32)
            # db = alpha*dens + beta ; alpha/beta as per-partition scalars
            alpha_rep = cpool.tile([NQ, 1], dt.float32)
            beta_rep = cpool.tile([NQ, 1], dt.float32)
            nc.gpsimd.partition_broadcast(alpha_rep[:], dcoef[0:1, 0:1])
            nc.gpsimd.partition_broadcast(beta_rep[:], dcoef[0:1, 1:2])
            nc.vector.tensor_scalar(out=db_rep[:], in0=dens_rep[:],
                                    scalar1=alpha_rep[:], scalar2=beta_rep[:],
                                    op0=ALU.mult, op1=ALU.add)

            # ---- scores + logsumexp ----
            sf = wpool.tile([NQ, N], dt.float32)
            z8 = spool.tile([NQ, NCH], dt.float32, tag="z8")
            with tc.tile_pool(name="psS", bufs=1, space="PSUM") as psS, \
                 tc.tile_pool(name="exps", bufs=2) as epool:
                s_tiles = []
                for c in range(NCH):
                    s_tiles.append(psS.tile([NQ, NCHUNK], dt.float32, tag=f"S{c}", name=f"S{c}"))
                xt_tiles = []
                for d in range(DCH):
                    xt = xpool.tile([128, N], dt.float32, tag="xt")
                    nc.sync.dma_start(xt[:, 0:N // 2],
                                      XTP.ap()[d * 128:(d + 1) * 128, 0:N // 2])
                    nc.sync.dma_start(xt[:, N // 2:N],
                                      XTP.ap()[d * 128:(d + 1) * 128, N // 2:N])
                    xt_tiles.append(xt)
                    for c in range(NCH):
                        nc.tensor.matmul(s_tiles[c][:],
                                         qwt[:, d, :],
                                         xt[:, c * NCHUNK:(c + 1) * NCHUNK],
                                         start=(d == 0), stop=(d == DCH - 1))
                for c in range(NCH):
                    cs = slice(c * NCHUNK, (c + 1) * NCHUNK)
                    nc.vector.tensor_tensor(sf[:, cs], s_tiles[c][:],
                                            db_rep[:, cs], op=ALU.add)
                    e = epool.tile([NQ, NCHUNK], dt.float32, tag="e")
                    nc.scalar.activation(e[:], sf[:, cs], AF.Exp,
                                         accum_out=z8[:, c:c + 1])

            zs = spool.tile([NQ, 1], dt.float32, tag="zs")
            nc.vector.tensor_reduce(zs[:], z8[:], axis=mybir.AxisListType.X,
                                    op=ALU.add)
            cq = spool.tile([NQ, 1], dt.float32, tag="cq")
            nc.scalar.activation(cq[:], zs[:], AF.Ln)

            # ---- g = max_q (sf - C) ----
            nc.vector.tensor_scalar(out=sf[:], in0=sf[:], scalar1=cq[:],
                                    scalar2=None, op0=ALU.subtract)
            ar = wpool.tile([NQ, N], dt.float32)
            nc.gpsimd.partition_all_reduce(ar[:], sf[:], channels=NQ,
                                           reduce_op=bass_isa.ReduceOp.max)

            # ---- exact top-1024 threshold search (64-ary grid) ----
            # every partition of `ar` already holds the full g row; round
            # state [lo, w] is kept replicated across 64 partitions so the
            # only cross-partition step is an all-reduce-add of the 64
            # bracket indicators.
            strep = spool.tile([64, 2], dt.float32, tag="strep")
            st0 = spool.tile([1, 2], dt.float32, tag="st0")
            nc.sync.dma_start(st0[:], STATE0.ap())
            nc.gpsimd.partition_broadcast(strep[:], st0[:])
            lo_rep = strep[:, 0:1]
            w_rep = strep[:, 1:2]
            scratch = wpool.tile([64, N], dt.float32)
            thr = spool.tile([64, 1], dt.float32, tag="thr")
            cnt = spool.tile([64, 1], dt.float32, tag="cnt")
            cge = spool.tile([64, 1], dt.float32, tag="cge")
            # thr_p = lo + (p+1) * w
            nc.vector.scalar_tensor_tensor(out=thr[:], in0=iotac[:],
                                           scalar=w_rep, in1=lo_rep,
                                           op0=ALU.mult, op1=ALU.add)
            with tc.tile_pool(name="psT", bufs=2, space="PSUM") as psT:
                for r in range(ROUNDS):
                    nc.vector.tensor_scalar(out=scratch[:], in0=ar[0:64, :],
                                            scalar1=thr[:], scalar2=0.0,
                                            op0=ALU.is_ge, op1=ALU.add,
                                            accum_out=cnt[:])
                    nc.vector.tensor_scalar(out=cge[:], in0=cnt[:],
                                            scalar1=float(TOPK), scalar2=None,
                                            op0=ALU.is_ge)
                    # replicated partition-sum of the 64 indicators via all-ones
                    # matmul (keeps gpsimd free for the library preload)
                    psr = psT.tile([64, 1], dt.float32, tag="psr", name=f"psr{r}")
                    nc.tensor.matmul(psr[:], j64[:], cge[:], start=True, stop=True)
                    # lo += pstar * w ; w /= 64 ; thr = lo + (p+1) * w
                    nc.vector.scalar_tensor_tensor(out=lo_rep, in0=psr[:],
                                                   scalar=w_rep, in1=lo_rep,
                                                   op0=ALU.mult, op1=ALU.add)
                    nc.vector.tensor_scalar(out=w_rep, in0=w_rep,
                                            scalar1=1.0 / 64.0, scalar2=None,
                                            op0=ALU.mult)
                    if r < ROUNDS - 1:
                        nc.vector.scalar_tensor_tensor(out=thr[:], in0=iotac[:],
                                                       scalar=w_rep, in1=lo_rep,
                                                       op0=ALU.mult, op1=ALU.add)
                # final threshold t = lo (replicated in lo_rep)

                # ---- compaction: masked iota of ids -> sparse_gather ----
                g16 = spool.tile([16, 256], dt.float32, tag="g16")
                nc.sync.dma_start(g16[:], ar[0:1, :].rearrange("o (r m) -> o r m",
                                                               r=16, m=256))
                mge = spool.tile([16, 256], dt.float32, tag="mge")
                nc.vector.tensor_scalar(out=mge[:], in0=g16[:],
                                        scalar1=lo_rep[0:16, :],
                                        scalar2=None, op0=ALU.is_ge)
                m16 = spool.tile([16, 256], dt.float32, tag="m16")
                nc.vector.tensor_tensor(m16[:], mge[:], iota16[:], op=ALU.mult)
                nc.vector.tensor_scalar(out=m16[:], in0=m16[:], scalar1=-1.0,
                                        scalar2=None, op0=ALU.add)
                comp = spool.tile([16, TOPK // 16], dt.float32, tag="comp")
                nfound = spool.tile([1, 1], dt.uint32, tag="nf")
                nc.gpsimd.sparse_gather(comp[:], m16[:], num_found=nfound[:])

                # ---- selected ids to [128, 8] int32 (k = 8p + c order) ----
                ct = psT.tile([64, 16], dt.float32, tag="ct")
                nc.tensor.transpose(ct[:], comp[:], id16[:])
                cti = spool.tile([64, 16], dt.int32, tag="cti")
                nc.vector.tensor_copy(cti[:], ct[:])
                ctib = spool.tile([128, 8], dt.int32, tag="ctib")
                nc.sync.dma_start(
                    ctib[:],
                    cti[:].rearrange("p (b c) -> p b c", b=2, c=8))
            for h in range(4):
                gt = gpool.tile([128, 2, D], dt.float32, tag="gt", name=f"gt{h}")
                for j in range(2):
                    f = 2 * h + j
                    nc.gpsimd.indirect_dma_start(
                        out=gt[:, j, :], out_offset=None, in_=X.ap(),
                        in_offset=bass.IndirectOffsetOnAxis(ap=ctib[:, f:f + 1],
                                                            axis=0))
                dst = OUT.ap().rearrange("(p f) d -> p f d", p=128,
                                         f=8)[:, 2 * h:2 * (h + 1), :]
                nc.sync.dma_start(dst, gt[:])
    nc.compile()
    return nc


_NC_CACHE = None


def _get_nc():
    global _NC_CACHE
    if _NC_CACHE is None:
        _NC_CACHE = _build_bass()
    return _NC_CACHE


def kernel(token_features, token_densities, query_embed,
           key_w, key_b, de_w1, de_b1, de_w2, de_b2):
    from concourse import bass_utils

    X = np.ascontiguousarray(np.asarray(token_features, dtype=np.float32))
    dens = np.asarray(token_densities, dtype=np.float32)
    Q64 = np.asarray(query_embed, dtype=np.float64)
    kw64 = np.asarray(key_w, dtype=np.float64)
    w1 = np.asarray(de_w1, dtype=np.float64)
    b1 = np.asarray(de_b1, dtype=np.float64)
    w2 = np.asarray(de_w2, dtype=np.float64)
    b2 = np.asarray(de_b2, dtype=np.float64)

    # QWT[d, q] = (query_embed @ key_w^T / sqrt(H))^T  (key_b cancels in softmax)
    QWT = ((Q64 @ kw64.T) / np.sqrt(np.float64(H))).T.astype(np.float32)
    QWT = np.ascontiguousarray(QWT)

    # density bias: exact linear collapse when b1 == 0 and d > 0, else host MLP
    linear_ok = np.all(b1 == 0.0) and np.all(dens > 0.0)
    if linear_ok:
        alpha = float(np.maximum(w1[0], 0.0) @ w2[:, 0])
        beta = float(b2[0])
        dens_dev = dens                       # device computes alpha*d + beta
    else:
        hm = np.maximum(dens[..., None].astype(np.float64) @ w1 + b1, 0.0)
        dens_dev = ((hm @ w2 + b2)[..., 0]).astype(np.float32)  # = db itself
        alpha, beta = 1.0, 0.0

    # device token permutation: position k holds token n = (k%256)*16 + k//256
    # XTp = X^T with columns permuted; built as reshape/transpose
    perm_cols = lambda a: np.ascontiguousarray(
        a.reshape(a.shape[0], 256, 16).transpose(0, 2, 1).reshape(a.shape[0], N))

    iota16 = (np.arange(256, dtype=np.float32)[None, :] * 16.0
              + np.arange(16, dtype=np.float32)[:, None] + 1.0)  # original id + 1
    iotac = (1.0 + np.arange(64, dtype=np.float32)).reshape(64, 1)
    w0 = (HI0 - LO0) / 64.0
    state0 = np.array([[LO0, w0]], np.float32)
    j64 = np.ones((64, 64), np.float32)
    ident16 = np.eye(16, dtype=np.float32)
    dcoefs = np.array([[alpha, beta]], np.float32)

    nc = _get_nc()
    in_maps = []
    for b in range(B):
        XT = np.ascontiguousarray(X[b].T)                     # [D, N]
        in_maps.append({
            "XTP": perm_cols(XT),
            "X": X[b],
            "DENS": perm_cols(dens_dev[b][None, :]),
            "QWT": QWT,
            "DCOEF": dcoefs,
            "IOTA16": iota16,
            "IOTAC": iotac,
            "STATE0": state0,
            "J64": j64,
            "ID16": ident16,
        })

    global _LAST_IN_MAPS
    _LAST_IN_MAPS = in_maps
    res = bass_utils.run_bass_kernel_spmd(nc, in_maps, core_ids=list(range(NC_COUNT)))
    out = np.stack([res.results[b]["OUT"] for b in range(B)])
    return out.astype(np.float32)


_LAST_IN_MAPS = None


# revision 19
# speedup vs baseline: 1.1622x; 1.0301x over previous
"""DensityGuidedCompressor Trainium2 kernel.

Problem: B=8, N=4096, D=1024, H=1024, NQ=64, TOPK=1024.
  K = X @ key_w + key_b                       [B,N,H]
  s = (query_embed @ K^T)/sqrt(H) + db[n]     [B,NQ,N]
  w = softmax(s, axis=-1); imp = max_q w      [B,N]
  idx = sort(top_k(imp, 1024))                [B,1024]
  out = X[idx]                                [B,1024,D]

Strategy (data-parallel, one batch element per NeuronCore):

Math reductions (exact in real arithmetic, fp32 on device):
  * key_b shifts every score of a q-row uniformly -> cancels in softmax and
    in importance; dropped.
  * q @ K^T = (q @ key_w^T) @ X^T: precompute QW = query_embed @ key_w^T / 32
    on host (param-only transform, 64x1024) -> the big matmul contracts over
    D once instead of computing K at all (16x fewer FLOPs).
  * ranking by imp[n] = max_q softmax(s)[q,n] == ranking by
    g[n] = max_q (s[q,n] - C_q), C_q = logsumexp_n s[q,n]  (exp is monotone).
  * density MLP with b1==0 and d>0 collapses to db = alpha*d + b2 with
    alpha = sum relu(w1)*w2 (host-checked; exact fallback: host computes db).

Device pipeline per core (all selection + gather on device):
  1. S[64,4096] = QWT^T @ XTp in PSUM (fp32 matmuls, 8x8 tiles of K=128,N=512)
  2. SF = S + db_rep; z_q = sum_n exp(SF) (ACT accum); C = ln(z)
  3. SC = SF - C; g = partition_all_reduce_max over the 64 q rows -> [1,4096]
  4. exact 1024th-largest threshold: 4 rounds of 128-way counting grid
     (PE broadcast matmul for the per-partition threshold vector, DVE
     is_ge+accum counts on a partition-broadcast copy of g, PE ones-matmul
     partition reduction, scalar update of [lo,w] on partition 0)
  5. mask = g >= t; masked iota of ORIGINAL token ids; gpsimd sparse_gather
     stream-compacts the 1024 selected ids in ascending order
  6. gpsimd dma_gather pulls the 1024 selected X rows (4KB each) from HBM,
     one DMA writes them back to OUT in row order.

Token permutation: device free-dim position k holds original token
n = (k%256)*16 + k//256 so that the [1,4096] g row splits contiguously into
the [16,256] (partition-minor) layout sparse_gather scans, with the scan
order equal to ascending original token id. XT columns and densities are
permuted on host; the masked-iota constant carries original ids.
"""

import numpy as np

B, N, D, H, NQ = 8, 4096, 1024, 1024, 64
TOPK = 1024
NCHUNK = 512          # matmul moving-operand free size (fp32 max)
NC_COUNT = 8          # cores
ROUNDS = 4            # threshold grid refinement rounds (64-ary)
LO0, HI0 = -16.0, 2.0  # conservative initial g bracket (g in ~[-12,-4] for
                       # unit-scale inputs; 18/64^4 = 1.1e-6 bracket width vs
                       # observed min rank-1024/1025 gap 5.8e-5)


def _build_bass():
    import concourse.bacc as bacc
    import concourse.mybir as mybir
    import concourse.tile as tile
    import concourse.bass as bass
    from concourse import bass_isa

    dt = mybir.dt
    ALU = mybir.AluOpType
    AF = mybir.ActivationFunctionType

    nc = bacc.Bacc("TRN2", target_bir_lowering=False, debug=False)

    XTP = nc.dram_tensor("XTP", [D, N], dt.float32, kind="ExternalInput")
    X = nc.dram_tensor("X", [N, D], dt.float32, kind="ExternalInput")
    DENS = nc.dram_tensor("DENS", [1, N], dt.float32, kind="ExternalInput")
    QWT = nc.dram_tensor("QWT", [D, NQ], dt.float32, kind="ExternalInput")
    DCOEF = nc.dram_tensor("DCOEF", [1, 2], dt.float32, kind="ExternalInput")
    IOTA16 = nc.dram_tensor("IOTA16", [16, 256], dt.float32, kind="ExternalInput")
    IOTAC = nc.dram_tensor("IOTAC", [64, 1], dt.float32, kind="ExternalInput")
    STATE0 = nc.dram_tensor("STATE0", [1, 2], dt.float32, kind="ExternalInput")
    J64 = nc.dram_tensor("J64", [64, 64], dt.float32, kind="ExternalInput")
    J2 = nc.dram_tensor("J2", [128, 64], dt.float32, kind="ExternalInput")
    JR = nc.dram_tensor("JR", [64, 128], dt.float32, kind="ExternalInput")
    IOTAC128 = nc.dram_tensor("IOTAC128", [128, 1], dt.float32, kind="ExternalInput")
    ID16 = nc.dram_tensor("ID16", [16, 16], dt.float32, kind="ExternalInput")
    OUT = nc.dram_tensor("OUT", [TOPK, D], dt.float32, kind="ExternalOutput")

    NCH = N // NCHUNK   # 8 n-chunks
    DCH = D // 128      # 8 d-chunks

    with tile.TileContext(nc) as tc:
        with tc.tile_pool(name="consts", bufs=1) as cpool, \
             tc.tile_pool(name="xtp", bufs=3) as xpool, \
             tc.tile_pool(name="work", bufs=1) as wpool, \
             tc.tile_pool(name="small", bufs=2) as spool, \
             tc.tile_pool(name="gath", bufs=8) as gpool:

            # ---- constants / params ----
            qwt = cpool.tile([128, DCH, NQ], dt.float32)
            nc.sync.dma_start(qwt[:], QWT.ap().rearrange("(c p) q -> p c q", c=DCH, p=128))
            dens = cpool.tile([1, N], dt.float32)
            nc.scalar.dma_start(dens[:], DENS.ap())
            dcoef = cpool.tile([1, 2], dt.float32)
            nc.scalar.dma_start(dcoef[:], DCOEF.ap())
            iota16 = cpool.tile([16, 256], dt.float32)
            nc.scalar.dma_start(iota16[:], IOTA16.ap())
            iotac = cpool.tile([64, 1], dt.float32)
            nc.scalar.dma_start(iotac[:], IOTAC.ap())
            j64 = cpool.tile([64, 64], dt.float32)
            nc.scalar.dma_start(j64[:], J64.ap())
            j2 = cpool.tile([128, 64], dt.float32)
            nc.scalar.dma_start(j2[:], J2.ap())
            jr = cpool.tile([64, 128], dt.float32)
            nc.scalar.dma_start(jr[:], JR.ap())
            iotac128 = cpool.tile([128, 1], dt.float32)
            nc.scalar.dma_start(iotac128[:], IOTAC128.ap())
            id16 = cpool.tile([16, 16], dt.float32)
            nc.scalar.dma_start(id16[:], ID16.ap())

            # ---- density bias, replicated across the 64 q partitions ----
            dens_rep = wpool.tile([NQ, N], dt.float32)
            nc.gpsimd.partition_broadcast(dens_rep[:], dens[:])
            db_rep = wpool.tile([NQ, N], dt.float32)
            # db = alpha*dens + beta ; alpha/beta as per-partition scalars
            alpha_rep = cpool.tile([NQ, 1], dt.float32)
            beta_rep = cpool.tile([NQ, 1], dt.float32)
            nc.gpsimd.partition_broadcast(alpha_rep[:], dcoef[0:1, 0:1])
            nc.gpsimd.partition_broadcast(beta_rep[:], dcoef[0:1, 1:2])
            nc.vector.tensor_scalar(out=db_rep[:], in0=dens_rep[:],
                                    scalar1=alpha_rep[:], scalar2=beta_rep[:],
                                    op0=ALU.mult, op1=ALU.add)

            # ---- scores + logsumexp ----
            sf = wpool.tile([NQ, N], dt.float32)
            z8 = spool.tile([NQ, NCH], dt.float32, tag="z8")
            with tc.tile_pool(name="psS", bufs=1, space="PSUM") as psS, \
                 tc.tile_pool(name="exps", bufs=2) as epool:
                s_tiles = []
                for c in range(NCH):
                    s_tiles.append(psS.tile([NQ, NCHUNK], dt.float32, tag=f"S{c}", name=f"S{c}"))
                xt_tiles = []
                for d in range(DCH):
                    xt = xpool.tile([128, N], dt.float32, tag="xt")
                    nc.sync.dma_start(xt[:, 0:N // 2],
                                      XTP.ap()[d * 128:(d + 1) * 128, 0:N // 2])
                    nc.sync.dma_start(xt[:, N // 2:N],
                                      XTP.ap()[d * 128:(d + 1) * 128, N // 2:N])
                    xt_tiles.append(xt)
                    for c in range(NCH):
                        nc.tensor.matmul(s_tiles[c][:],
                                         qwt[:, d, :],
                                         xt[:, c * NCHUNK:(c + 1) * NCHUNK],
                                         start=(d == 0), stop=(d == DCH - 1))
                for c in range(NCH):
                    cs = slice(c * NCHUNK, (c + 1) * NCHUNK)
                    nc.vector.tensor_tensor(sf[:, cs], s_tiles[c][:],
                                            db_rep[:, cs], op=ALU.add)
                    e = epool.tile([NQ, NCHUNK], dt.float32, tag="e")
                    nc.scalar.activation(e[:], sf[:, cs], AF.Exp,
                                         accum_out=z8[:, c:c + 1])

            zs = spool.tile([NQ, 1], dt.float32, tag="zs")
            nc.vector.tensor_reduce(zs[:], z8[:], axis=mybir.AxisListType.X,
                                    op=ALU.add)
            cq = spool.tile([NQ, 1], dt.float32, tag="cq")
            nc.scalar.activation(cq[:], zs[:], AF.Ln)

            # ---- g = max_q (sf - C) ----
            nc.vector.tensor_scalar(out=sf[:], in0=sf[:], scalar1=cq[:],
                                    scalar2=None, op0=ALU.subtract)
            ar = wpool.tile([NQ, N], dt.float32)
            nc.gpsimd.partition_all_reduce(ar[:], sf[:], channels=NQ,
                                           reduce_op=bass_isa.ReduceOp.max)

            # ---- exact top-1024 threshold search (64-ary grid) ----
            # spread g over all 128 DVE lanes: partitions p and p+64 hold the
            # two halves of g and test the same threshold; a fold matmul (J2)
            # recombines the two partial counts.
            strep = spool.tile([128, 2], dt.float32, tag="strep")
            st0 = spool.tile([1, 2], dt.float32, tag="st0")
            nc.sync.dma_start(st0[:], STATE0.ap())
            nc.gpsimd.partition_broadcast(strep[:], st0[:])
            lo_rep = strep[:, 0:1]
            w_rep = strep[:, 1:2]
            ar128 = wpool.tile([128, N // 2], dt.float32)
            nc.sync.dma_start(ar128[0:64, :], ar[0:64, 0:N // 2])
            nc.sync.dma_start(ar128[64:128, :], ar[0:64, N // 2:N])
            scratch = wpool.tile([128, N // 2], dt.float32)
            thr = spool.tile([128, 1], dt.float32, tag="thr")
            cnt = spool.tile([128, 1], dt.float32, tag="cnt")
            cge = spool.tile([64, 1], dt.float32, tag="cge")
            # thr_p = lo + ((p % 64) + 1) * w
            nc.vector.scalar_tensor_tensor(out=thr[:], in0=iotac128[:],
                                           scalar=w_rep, in1=lo_rep,
                                           op0=ALU.mult, op1=ALU.add)
            with tc.tile_pool(name="psT", bufs=2, space="PSUM") as psT:
                for r in range(ROUNDS):
                    nc.vector.tensor_scalar(out=scratch[:], in0=ar128[:],
                                            scalar1=thr[:], scalar2=0.0,
                                            op0=ALU.is_ge, op1=ALU.add,
                                            accum_out=cnt[:])
                    cnt64 = psT.tile([64, 1], dt.float32, tag="cnt64",
                                     name=f"cnt64_{r}")
                    nc.tensor.matmul(cnt64[:], j2[:], cnt[:], start=True, stop=True)
                    nc.vector.tensor_scalar(out=cge[:], in0=cnt64[:],
                                            scalar1=float(TOPK), scalar2=None,
                                            op0=ALU.is_ge)
                    psr = psT.tile([128, 1], dt.float32, tag="psr", name=f"psr{r}")
                    nc.tensor.matmul(psr[:], jr[:], cge[:], start=True, stop=True)
                    # lo += pstar * w ; w /= 64 ; thr = lo + ((p%64)+1) * w
                    nc.vector.scalar_tensor_tensor(out=lo_rep, in0=psr[:],
                                                   scalar=w_rep, in1=lo_rep,
                                                   op0=ALU.mult, op1=ALU.add)
                    nc.vector.tensor_scalar(out=w_rep, in0=w_rep,
                                            scalar1=1.0 / 64.0, scalar2=None,
                                            op0=ALU.mult)
                    if r < ROUNDS - 1:
                        nc.vector.scalar_tensor_tensor(out=thr[:], in0=iotac128[:],
                                                       scalar=w_rep, in1=lo_rep,
                                                       op0=ALU.mult, op1=ALU.add)
                # final threshold t = lo (replicated in lo_rep)

                # ---- compaction: masked iota of ids -> sparse_gather ----
                g16 = spool.tile([16, 256], dt.float32, tag="g16")
                nc.sync.dma_start(g16[:], ar[0:1, :].rearrange("o (r m) -> o r m",
                                                               r=16, m=256))
                mge = spool.tile([16, 256], dt.float32, tag="mge")
                nc.vector.tensor_scalar(out=mge[:], in0=g16[:],
                                        scalar1=lo_rep[0:16, :],
                                        scalar2=None, op0=ALU.is_ge)
                m16 = spool.tile([16, 256], dt.float32, tag="m16")
                nc.vector.tensor_tensor(m16[:], mge[:], iota16[:], op=ALU.mult)
                nc.vector.tensor_scalar(out=m16[:], in0=m16[:], scalar1=-1.0,
                                        scalar2=None, op0=ALU.add)
                comp = spool.tile([16, TOPK // 16], dt.float32, tag="comp")
                nfound = spool.tile([1, 1], dt.uint32, tag="nf")
                nc.gpsimd.sparse_gather(comp[:], m16[:], num_found=nfound[:])

                # ---- selected ids to [128, 8] int32 (k = 8p + c order) ----
                ct = psT.tile([64, 16], dt.float32, tag="ct")
                nc.tensor.transpose(ct[:], comp[:], id16[:])
                cti = spool.tile([64, 16], dt.int32, tag="cti")
                nc.vector.tensor_copy(cti[:], ct[:])
                ctib = spool.tile([128, 8], dt.int32, tag="ctib")
                nc.sync.dma_start(
                    ctib[:],
                    cti[:].rearrange("p (b c) -> p b c", b=2, c=8))
            for f in range(8):
                gt = gpool.tile([128, D], dt.float32, tag="gt", name=f"gt{f}")
                nc.gpsimd.indirect_dma_start(
                    out=gt[:], out_offset=None, in_=X.ap(),
                    in_offset=bass.IndirectOffsetOnAxis(ap=ctib[:, f:f + 1],
                                                        axis=0))
                dst = OUT.ap().rearrange("(p f) d -> p f d", p=128,
                                         f=8)[:, f:f + 1, :]
                nc.sync.dma_start(dst, gt[:].unsqueeze(1))
    nc.compile()
    return nc


_NC_CACHE = None


def _get_nc():
    global _NC_CACHE
    if _NC_CACHE is None:
        _NC_CACHE = _build_bass()
    return _NC_CACHE


def kernel(token_features, token_densities, query_embed,
           key_w, key_b, de_w1, de_b1, de_w2, de_b2):
    from concourse import bass_utils

    X = np.ascontiguousarray(np.asarray(token_features, dtype=np.float32))
    dens = np.asarray(token_densities, dtype=np.float32)
    Q64 = np.asarray(query_embed, dtype=np.float64)
    kw64 = np.asarray(key_w, dtype=np.float64)
    w1 = np.asarray(de_w1, dtype=np.float64)
    b1 = np.asarray(de_b1, dtype=np.float64)
    w2 = np.asarray(de_w2, dtype=np.float64)
    b2 = np.asarray(de_b2, dtype=np.float64)

    # QWT[d, q] = (query_embed @ key_w^T / sqrt(H))^T  (key_b cancels in softmax)
    QWT = ((Q64 @ kw64.T) / np.sqrt(np.float64(H))).T.astype(np.float32)
    QWT = np.ascontiguousarray(QWT)

    # density bias: exact linear collapse when b1 == 0 and d > 0, else host MLP
    linear_ok = np.all(b1 == 0.0) and np.all(dens > 0.0)
    if linear_ok:
        alpha = float(np.maximum(w1[0], 0.0) @ w2[:, 0])
        beta = float(b2[0])
        dens_dev = dens                       # device computes alpha*d + beta
    else:
        hm = np.maximum(dens[..., None].astype(np.float64) @ w1 + b1, 0.0)
        dens_dev = ((hm @ w2 + b2)[..., 0]).astype(np.float32)  # = db itself
        alpha, beta = 1.0, 0.0

    # device token permutation: position k holds token n = (k%256)*16 + k//256
    # XTp = X^T with columns permuted; built as reshape/transpose
    perm_cols = lambda a: np.ascontiguousarray(
        a.reshape(a.shape[0], 256, 16).transpose(0, 2, 1).reshape(a.shape[0], N))

    iota16 = (np.arange(256, dtype=np.float32)[None, :] * 16.0
              + np.arange(16, dtype=np.float32)[:, None] + 1.0)  # original id + 1
    iotac = (1.0 + np.arange(64, dtype=np.float32)).reshape(64, 1)
    w0 = (HI0 - LO0) / 64.0
    state0 = np.array([[LO0, w0]], np.float32)
    j64 = np.ones((64, 64), np.float32)
    j2 = np.zeros((128, 64), np.float32)
    j2[np.arange(128), np.arange(128) % 64] = 1.0
    jr = np.ones((64, 128), np.float32)
    iotac128 = (1.0 + (np.arange(128) % 64).astype(np.float32)).reshape(128, 1)
    ident16 = np.eye(16, dtype=np.float32)
    dcoefs = np.array([[alpha, beta]], np.float32)

    nc = _get_nc()
    in_maps = []
    for b in range(B):
        XT = np.ascontiguousarray(X[b].T)                     # [D, N]
        in_maps.append({
            "XTP": perm_cols(XT),
            "X": X[b],
            "DENS": perm_cols(dens_dev[b][None, :]),
            "QWT": QWT,
            "DCOEF": dcoefs,
            "IOTA16": iota16,
            "IOTAC": iotac,
            "STATE0": state0,
            "J64": j64,
            "J2": j2,
            "JR": jr,
            "IOTAC128": iotac128,
            "ID16": ident16,
        })

    global _LAST_IN_MAPS
    _LAST_IN_MAPS = in_maps
    res = bass_utils.run_bass_kernel_spmd(nc, in_maps, core_ids=list(range(NC_COUNT)))
    out = np.stack([res.results[b]["OUT"] for b in range(B)])
    return out.astype(np.float32)


_LAST_IN_MAPS = None


# revision 21
# speedup vs baseline: 1.1727x; 1.0090x over previous
"""DensityGuidedCompressor Trainium2 kernel.

Problem: B=8, N=4096, D=1024, H=1024, NQ=64, TOPK=1024.
  K = X @ key_w + key_b                       [B,N,H]
  s = (query_embed @ K^T)/sqrt(H) + db[n]     [B,NQ,N]
  w = softmax(s, axis=-1); imp = max_q w      [B,N]
  idx = sort(top_k(imp, 1024))                [B,1024]
  out = X[idx]                                [B,1024,D]

Strategy (data-parallel, one batch element per NeuronCore):

Math reductions (exact in real arithmetic, fp32 on device):
  * key_b shifts every score of a q-row uniformly -> cancels in softmax and
    in importance; dropped.
  * q @ K^T = (q @ key_w^T) @ X^T: precompute QW = query_embed @ key_w^T / 32
    on host (param-only transform, 64x1024) -> the big matmul contracts over
    D once instead of computing K at all (16x fewer FLOPs).
  * ranking by imp[n] = max_q softmax(s)[q,n] == ranking by
    g[n] = max_q (s[q,n] - C_q), C_q = logsumexp_n s[q,n]  (exp is monotone).
  * density MLP with b1==0 and d>0 collapses to db = alpha*d + b2 with
    alpha = sum relu(w1)*w2 (host-checked; exact fallback: host computes db).

Device pipeline per core (all selection + gather on device):
  1. S[64,4096] = QWT^T @ XTp in PSUM (fp32 matmuls, 8x8 tiles of K=128,N=512)
  2. SF = S + db_rep; z_q = sum_n exp(SF) (ACT accum); C = ln(z)
  3. SC = SF - C; g = partition_all_reduce_max over the 64 q rows -> [1,4096]
  4. exact 1024th-largest threshold: 4 rounds of 128-way counting grid
     (PE broadcast matmul for the per-partition threshold vector, DVE
     is_ge+accum counts on a partition-broadcast copy of g, PE ones-matmul
     partition reduction, scalar update of [lo,w] on partition 0)
  5. mask = g >= t; masked iota of ORIGINAL token ids; gpsimd sparse_gather
     stream-compacts the 1024 selected ids in ascending order
  6. gpsimd dma_gather pulls the 1024 selected X rows (4KB each) from HBM,
     one DMA writes them back to OUT in row order.

Token permutation: device free-dim position k holds original token
n = (k%256)*16 + k//256 so that the [1,4096] g row splits contiguously into
the [16,256] (partition-minor) layout sparse_gather scans, with the scan
order equal to ascending original token id. XT columns and densities are
permuted on host; the masked-iota constant carries original ids.
"""

import numpy as np

B, N, D, H, NQ = 8, 4096, 1024, 1024, 64
TOPK = 1024
NCHUNK = 512          # matmul moving-operand free size (fp32 max)
NC_COUNT = 8          # cores
ROUNDS = 4            # threshold grid refinement rounds (64-ary)
LO0, HI0 = -16.0, 2.0  # conservative initial g bracket (g in ~[-12,-4] for
                       # unit-scale inputs; 18/64^4 = 1.1e-6 bracket width vs
                       # observed min rank-1024/1025 gap 5.8e-5)


def _build_bass():
    import concourse.bacc as bacc
    import concourse.mybir as mybir
    import concourse.tile as tile
    import concourse.bass as bass
    from concourse import bass_isa

    dt = mybir.dt
    ALU = mybir.AluOpType
    AF = mybir.ActivationFunctionType

    nc = bacc.Bacc("TRN2", target_bir_lowering=False, debug=False)

    XTP = nc.dram_tensor("XTP", [D, N], dt.float32, kind="ExternalInput")
    X = nc.dram_tensor("X", [N, D], dt.float32, kind="ExternalInput")
    DENS = nc.dram_tensor("DENS", [1, N], dt.float32, kind="ExternalInput")
    QWT = nc.dram_tensor("QWT", [D, NQ], dt.float32, kind="ExternalInput")
    DCOEF = nc.dram_tensor("DCOEF", [1, 2], dt.float32, kind="ExternalInput")
    IOTA16 = nc.dram_tensor("IOTA16", [16, 256], dt.float32, kind="ExternalInput")
    IOTAC = nc.dram_tensor("IOTAC", [64, 1], dt.float32, kind="ExternalInput")
    STATE0 = nc.dram_tensor("STATE0", [1, 2], dt.float32, kind="ExternalInput")
    J64 = nc.dram_tensor("J64", [64, 64], dt.float32, kind="ExternalInput")
    J2 = nc.dram_tensor("J2", [128, 64], dt.float32, kind="ExternalInput")
    JR = nc.dram_tensor("JR", [64, 128], dt.float32, kind="ExternalInput")
    IOTAC128 = nc.dram_tensor("IOTAC128", [128, 1], dt.float32, kind="ExternalInput")
    ID16 = nc.dram_tensor("ID16", [16, 16], dt.float32, kind="ExternalInput")
    OUT = nc.dram_tensor("OUT", [TOPK, D], dt.float32, kind="ExternalOutput")

    NCH = N // NCHUNK   # 8 n-chunks
    DCH = D // 128      # 8 d-chunks

    with tile.TileContext(nc) as tc:
        with tc.tile_pool(name="consts", bufs=1) as cpool, \
             tc.tile_pool(name="xtp", bufs=3) as xpool, \
             tc.tile_pool(name="work", bufs=1) as wpool, \
             tc.tile_pool(name="small", bufs=2) as spool, \
             tc.tile_pool(name="gath", bufs=8) as gpool:

            # ---- constants / params ----
            qwt = cpool.tile([128, DCH, NQ], dt.float32)
            nc.sync.dma_start(qwt[:], QWT.ap().rearrange("(c p) q -> p c q", c=DCH, p=128))
            dens = cpool.tile([1, N], dt.float32)
            nc.scalar.dma_start(dens[:], DENS.ap())
            dcoef = cpool.tile([1, 2], dt.float32)
            nc.scalar.dma_start(dcoef[:], DCOEF.ap())
            iota16 = cpool.tile([16, 256], dt.float32)
            nc.scalar.dma_start(iota16[:], IOTA16.ap())
            iotac = cpool.tile([64, 1], dt.float32)
            nc.scalar.dma_start(iotac[:], IOTAC.ap())
            j64 = cpool.tile([64, 64], dt.float32)
            nc.scalar.dma_start(j64[:], J64.ap())
            j2 = cpool.tile([128, 64], dt.float32)
            nc.scalar.dma_start(j2[:], J2.ap())
            jr = cpool.tile([64, 128], dt.float32)
            nc.scalar.dma_start(jr[:], JR.ap())
            iotac128 = cpool.tile([128, 1], dt.float32)
            nc.scalar.dma_start(iotac128[:], IOTAC128.ap())
            id16 = cpool.tile([16, 16], dt.float32)
            nc.scalar.dma_start(id16[:], ID16.ap())

            # ---- density bias, replicated across the 64 q partitions ----
            dens_rep = wpool.tile([NQ, N], dt.float32)
            nc.gpsimd.partition_broadcast(dens_rep[:], dens[:])
            db_rep = wpool.tile([NQ, N], dt.float32)
            # db = alpha*dens + beta ; alpha/beta as per-partition scalars
            alpha_rep = cpool.tile([NQ, 1], dt.float32)
            beta_rep = cpool.tile([NQ, 1], dt.float32)
            nc.gpsimd.partition_broadcast(alpha_rep[:], dcoef[0:1, 0:1])
            nc.gpsimd.partition_broadcast(beta_rep[:], dcoef[0:1, 1:2])
            nc.vector.tensor_scalar(out=db_rep[:], in0=dens_rep[:],
                                    scalar1=alpha_rep[:], scalar2=beta_rep[:],
                                    op0=ALU.mult, op1=ALU.add)

            # ---- scores + logsumexp ----
            sf = wpool.tile([NQ, N], dt.float32)
            z8 = spool.tile([NQ, NCH], dt.float32, tag="z8")
            with tc.tile_pool(name="psS", bufs=1, space="PSUM") as psS, \
                 tc.tile_pool(name="exps", bufs=2) as epool:
                s_tiles = []
                for c in range(NCH):
                    s_tiles.append(psS.tile([NQ, NCHUNK], dt.float32, tag=f"S{c}", name=f"S{c}"))
                xt_tiles = []
                for d in range(DCH):
                    xt = xpool.tile([128, N], dt.float32, tag="xt")
                    nc.sync.dma_start(xt[:, 0:N // 2],
                                      XTP.ap()[d * 128:(d + 1) * 128, 0:N // 2])
                    nc.sync.dma_start(xt[:, N // 2:N],
                                      XTP.ap()[d * 128:(d + 1) * 128, N // 2:N])
                    xt_tiles.append(xt)
                    for c in range(NCH):
                        nc.tensor.matmul(s_tiles[c][:],
                                         qwt[:, d, :],
                                         xt[:, c * NCHUNK:(c + 1) * NCHUNK],
                                         start=(d == 0), stop=(d == DCH - 1))
                for c in range(NCH):
                    cs = slice(c * NCHUNK, (c + 1) * NCHUNK)
                    nc.vector.tensor_tensor(sf[:, cs], s_tiles[c][:],
                                            db_rep[:, cs], op=ALU.add)
                    e = epool.tile([NQ, NCHUNK], dt.float32, tag="e")
                    nc.scalar.activation(e[:], sf[:, cs], AF.Exp,
                                         accum_out=z8[:, c:c + 1])

            zs = spool.tile([NQ, 1], dt.float32, tag="zs")
            nc.vector.tensor_reduce(zs[:], z8[:], axis=mybir.AxisListType.X,
                                    op=ALU.add)
            cq = spool.tile([NQ, 1], dt.float32, tag="cq")
            nc.scalar.activation(cq[:], zs[:], AF.Ln)

            # ---- g = max_q (sf - C) ----
            nc.vector.tensor_scalar(out=sf[:], in0=sf[:], scalar1=cq[:],
                                    scalar2=None, op0=ALU.subtract)
            # two half-width all-reduces; the first writes its half of the
            # [128, N/2] lane-split layout directly, the second goes through a
            # temp tile (gpsimd cannot write at partition base 64)
            ar128 = wpool.tile([128, N // 2], dt.float32)
            artmp = wpool.tile([64, N // 2], dt.float32)
            nc.gpsimd.partition_all_reduce(ar128[0:64, :], sf[:, 0:N // 2],
                                           channels=NQ,
                                           reduce_op=bass_isa.ReduceOp.max)
            nc.gpsimd.partition_all_reduce(artmp[:], sf[:, N // 2:N],
                                           channels=NQ,
                                           reduce_op=bass_isa.ReduceOp.max)
            nc.sync.dma_start(ar128[64:128, :], artmp[:])

            # ---- exact top-1024 threshold search (64-ary grid) ----
            # spread g over all 128 DVE lanes: partitions p and p+64 hold the
            # two halves of g and test the same threshold; a fold matmul (J2)
            # recombines the two partial counts.
            strep = spool.tile([128, 2], dt.float32, tag="strep")
            st0 = spool.tile([1, 2], dt.float32, tag="st0")
            nc.sync.dma_start(st0[:], STATE0.ap())
            nc.gpsimd.partition_broadcast(strep[:], st0[:])
            lo_rep = strep[:, 0:1]
            w_rep = strep[:, 1:2]
            scratch = wpool.tile([128, N // 2], dt.float32)
            thr = spool.tile([128, 1], dt.float32, tag="thr")
            cnt = spool.tile([128, 1], dt.float32, tag="cnt")
            cge = spool.tile([64, 1], dt.float32, tag="cge")
            # thr_p = lo + ((p % 64) + 1) * w
            nc.vector.scalar_tensor_tensor(out=thr[:], in0=iotac128[:],
                                           scalar=w_rep, in1=lo_rep,
                                           op0=ALU.mult, op1=ALU.add)
            with tc.tile_pool(name="psT", bufs=2, space="PSUM") as psT:
                for r in range(ROUNDS):
                    nc.vector.tensor_scalar(out=scratch[:], in0=ar128[:],
                                            scalar1=thr[:], scalar2=0.0,
                                            op0=ALU.is_ge, op1=ALU.add,
                                            accum_out=cnt[:])
                    cnt64 = psT.tile([64, 1], dt.float32, tag="cnt64",
                                     name=f"cnt64_{r}")
                    nc.tensor.matmul(cnt64[:], j2[:], cnt[:], start=True, stop=True)
                    nc.vector.tensor_scalar(out=cge[:], in0=cnt64[:],
                                            scalar1=float(TOPK), scalar2=None,
                                            op0=ALU.is_ge)
                    psr = psT.tile([128, 1], dt.float32, tag="psr", name=f"psr{r}")
                    nc.tensor.matmul(psr[:], jr[:], cge[:], start=True, stop=True)
                    # lo += pstar * w ; w /= 64 ; thr = lo + ((p%64)+1) * w
                    nc.vector.scalar_tensor_tensor(out=lo_rep, in0=psr[:],
                                                   scalar=w_rep, in1=lo_rep,
                                                   op0=ALU.mult, op1=ALU.add)
                    nc.vector.tensor_scalar(out=w_rep, in0=w_rep,
                                            scalar1=1.0 / 64.0, scalar2=None,
                                            op0=ALU.mult)
                    if r < ROUNDS - 1:
                        nc.vector.scalar_tensor_tensor(out=thr[:], in0=iotac128[:],
                                                       scalar=w_rep, in1=lo_rep,
                                                       op0=ALU.mult, op1=ALU.add)
                # final threshold t = lo (replicated in lo_rep)

                # ---- compaction: masked iota of ids -> sparse_gather ----
                g16 = spool.tile([16, 256], dt.float32, tag="g16")
                nc.sync.dma_start(
                    g16[0:8, :],
                    ar128[0:1, :].rearrange("o (r m) -> o r m", r=8, m=256))
                nc.scalar.dma_start(
                    g16[8:16, :],
                    artmp[0:1, :].rearrange("o (r m) -> o r m", r=8, m=256))
                mge = spool.tile([16, 256], dt.float32, tag="mge")
                nc.vector.tensor_scalar(out=mge[:], in0=g16[:],
                                        scalar1=lo_rep[0:16, :],
                                        scalar2=None, op0=ALU.is_ge)
                m16 = spool.tile([16, 256], dt.float32, tag="m16")
                nc.vector.tensor_tensor(m16[:], mge[:], iota16[:], op=ALU.mult)
                nc.vector.tensor_scalar(out=m16[:], in0=m16[:], scalar1=-1.0,
                                        scalar2=None, op0=ALU.add)
                comp = spool.tile([16, TOPK // 16], dt.float32, tag="comp")
                nfound = spool.tile([1, 1], dt.uint32, tag="nf")
                nc.gpsimd.sparse_gather(comp[:], m16[:], num_found=nfound[:])

                # ---- selected ids to [128, 8] int32 (k = 8p + c order) ----
                ct = psT.tile([64, 16], dt.float32, tag="ct")
                nc.tensor.transpose(ct[:], comp[:], id16[:])
                cti = spool.tile([64, 16], dt.int32, tag="cti")
                nc.vector.tensor_copy(cti[:], ct[:])
                ctib = spool.tile([128, 8], dt.int32, tag="ctib")
                nc.sync.dma_start(
                    ctib[:],
                    cti[:].rearrange("p (b c) -> p b c", b=2, c=8))
            for f in range(8):
                gt = gpool.tile([128, D], dt.float32, tag="gt", name=f"gt{f}")
                nc.gpsimd.indirect_dma_start(
                    out=gt[:], out_offset=None, in_=X.ap(),
                    in_offset=bass.IndirectOffsetOnAxis(ap=ctib[:, f:f + 1],
                                                        axis=0))
                dst = OUT.ap().rearrange("(p f) d -> p f d", p=128,
                                         f=8)[:, f:f + 1, :]
                nc.sync.dma_start(dst, gt[:].unsqueeze(1))
    nc.compile()
    return nc


_NC_CACHE = None


def _get_nc():
    global _NC_CACHE
    if _NC_CACHE is None:
        _NC_CACHE = _build_bass()
    return _NC_CACHE


def kernel(token_features, token_densities, query_embed,
           key_w, key_b, de_w1, de_b1, de_w2, de_b2):
    from concourse import bass_utils

    X = np.ascontiguousarray(np.asarray(token_features, dtype=np.float32))
    dens = np.asarray(token_densities, dtype=np.float32)
    Q64 = np.asarray(query_embed, dtype=np.float64)
    kw64 = np.asarray(key_w, dtype=np.float64)
    w1 = np.asarray(de_w1, dtype=np.float64)
    b1 = np.asarray(de_b1, dtype=np.float64)
    w2 = np.asarray(de_w2, dtype=np.float64)
    b2 = np.asarray(de_b2, dtype=np.float64)

    # QWT[d, q] = (query_embed @ key_w^T / sqrt(H))^T  (key_b cancels in softmax)
    QWT = ((Q64 @ kw64.T) / np.sqrt(np.float64(H))).T.astype(np.float32)
    QWT = np.ascontiguousarray(QWT)

    # density bias: exact linear collapse when b1 == 0 and d > 0, else host MLP
    linear_ok = np.all(b1 == 0.0) and np.all(dens > 0.0)
    if linear_ok:
        alpha = float(np.maximum(w1[0], 0.0) @ w2[:, 0])
        beta = float(b2[0])
        dens_dev = dens                       # device computes alpha*d + beta
    else:
        hm = np.maximum(dens[..., None].astype(np.float64) @ w1 + b1, 0.0)
        dens_dev = ((hm @ w2 + b2)[..., 0]).astype(np.float32)  # = db itself
        alpha, beta = 1.0, 0.0

    # device token permutation: position k holds token n = (k%256)*16 + k//256
    # XTp = X^T with columns permuted; built as reshape/transpose
    perm_cols = lambda a: np.ascontiguousarray(
        a.reshape(a.shape[0], 256, 16).transpose(0, 2, 1).reshape(a.shape[0], N))

    iota16 = (np.arange(256, dtype=np.float32)[None, :] * 16.0
              + np.arange(16, dtype=np.float32)[:, None] + 1.0)  # original id + 1
    iotac = (1.0 + np.arange(64, dtype=np.float32)).reshape(64, 1)
    w0 = (HI0 - LO0) / 64.0
    state0 = np.array([[LO0, w0]], np.float32)
    j64 = np.ones((64, 64), np.float32)
    j2 = np.zeros((128, 64), np.float32)
    j2[np.arange(128), np.arange(128) % 64] = 1.0
    jr = np.ones((64, 128), np.float32)
    iotac128 = (1.0 + (np.arange(128) % 64).astype(np.float32)).reshape(128, 1)
    ident16 = np.eye(16, dtype=np.float32)
    dcoefs = np.array([[alpha, beta]], np.float32)

    nc = _get_nc()
    in_maps = []
    for b in range(B):
        XT = np.ascontiguousarray(X[b].T)                     # [D, N]
        in_maps.append({
            "XTP": perm_cols(XT),
            "X": X[b],
            "DENS": perm_cols(dens_dev[b][None, :]),
            "QWT": QWT,
            "DCOEF": dcoefs,
            "IOTA16": iota16,
            "IOTAC": iotac,
            "STATE0": state0,
            "J64": j64,
            "J2": j2,
            "JR": jr,
            "IOTAC128": iotac128,
            "ID16": ident16,
        })

    global _LAST_IN_MAPS
    _LAST_IN_MAPS = in_maps
    res = bass_utils.run_bass_kernel_spmd(nc, in_maps, core_ids=list(range(NC_COUNT)))
    out = np.stack([res.results[b]["OUT"] for b in range(B)])
    return out.astype(np.float32)


_LAST_IN_MAPS = None
